# revision 21
# baseline (speedup 1.0000x reference)
"""Character-LSTM Trainium2 kernel (8 NeuronCores, SPMD data-parallel).

Strategy
--------
All B*S = 16384 words run one batched LSTM recurrence. Work is split across 8
cores by dealing words (sorted by descending length) round-robin so every core
sees an identical per-step active-column count A[t]; within a core, words live
in SBUF as columns of transposed state tiles [H x cols]. At step t only the
first A[t] columns are touched, so a word's last update lands exactly at its
final character and the surviving h columns are the output. Short word-length
buckets are padded with dummy columns (char 0 -> zero embedding row) so A[t]
is core-uniform and a multiple of 16.

Per step, gates are computed in transposed layout g[4H x cols] on the PE as
one accumulation over four K=128 chunks: two one-hot chunks against the
per-vocab gate table emb_proj = W_ih @ emb[v] (precomputed on device), and two
h chunks against W_hh - all bf16 with fp32 PSUM accumulation. One-hots are
built on device by GPSIMD is_equal against a per-partition iota, from a
DMA-broadcast char row. Sigmoid/tanh run on the scalar engine straight out of
1024-wide PSUM reads with the fused per-partition bias. The cell update runs
on the vector engine in fp32; h keeps an fp32 master copy (the output) and a
GPSIMD-converted bf16 copy that feeds the next step's matmuls.
"""

import sys

if "/opt/trn_rl_repo" not in sys.path:
    sys.path.insert(0, "/opt/trn_rl_repo")

import contextlib

import numpy as np
import ml_dtypes

import concourse.bass as bass
import concourse.tile as tile
from concourse import bacc, mybir
from concourse.bass import ts
from concourse.bass_utils import run_bass_kernel_spmd

BF16 = ml_dtypes.bfloat16
NCORES = 8
B, S, W, E, H, V = 64, 256, 24, 128, 256, 256
GATE_FUNCS = ["Sigmoid", "Sigmoid", "Tanh", "Sigmoid"]  # i, f, g, o per 2 chunks
QW = 1024  # PSUM tile width (2 banks); ACT reads PSUM at line rate at this width
MM = 512  # matmul moving free-dim

_PROGRAM_CACHE: dict = {}


def _plan(lens: np.ndarray):
    """Column counts per step, uniform across cores, multiples of 16."""
    wL = np.bincount(lens, minlength=W + 1)
    colsL = np.zeros(W + 1, np.int64)
    cum = 0
    for L in range(W, 0, -1):
        need = -(-int(wL[L]) // NCORES)
        newcum = -(-(cum + need) // 16) * 16
        colsL[L] = newcum - cum
        cum = newcum
    C = max(cum, 16)
    A = [int(colsL[t + 1 :].sum()) for t in range(W)]
    return colsL, C, A


def _assign(lens, chars, colsL, C):
    """Deal words into (core, column) slots, longest first."""
    order = np.argsort(-lens, kind="stable")
    wL = np.bincount(lens, minlength=W + 1)
    colmap = np.full((NCORES, C), -1, np.int64)
    col_chars = np.zeros((NCORES, C, W), np.int64)
    pos = 0
    s = 0
    for L in range(W, 0, -1):
        cnt = int(wL[L])
        if cnt:
            ids = order[pos : pos + cnt]
            pos += cnt
            k = np.arange(cnt) % NCORES
            j = s + np.arange(cnt) // NCORES
            colmap[k, j] = ids
            col_chars[k, j] = chars[ids]
        s += int(colsL[L])
    return colmap, col_chars


def _build_program(C: int, A: list[int], reps: int = 1, act_pad: bool = True, hcopy: str = 'gpsimd', cell_sub: int = 0, chrep_host: bool = False, nochain: bool = False, mm_n: int = 512, act_round: int = 0, x_sep: bool = False):
    key = (C, tuple(A), reps, act_pad, hcopy, cell_sub, chrep_host, nochain, mm_n, act_round, x_sep)
    if key in _PROGRAM_CACHE:
        return _PROGRAM_CACHE[key]

    dt = mybir.dt
    AF = mybir.ActivationFunctionType
    EQ = mybir.AluOpType.is_equal
    nc = bacc.Bacc("TRN2", target_bir_lowering=False, debug=False, num_devices=NCORES)

    if chrep_host:
        chf_d = nc.dram_tensor("chfr", [W, 128, C], dt.bfloat16, kind="ExternalInput")
    else:
        chf_d = nc.dram_tensor("chf", [W, C], dt.bfloat16, kind="ExternalInput")
    embt_d = nc.dram_tensor("embt", [128, V], dt.bfloat16, kind="ExternalInput")
    wih_d = nc.dram_tensor("wih", [E, 4 * H], dt.bfloat16, kind="ExternalInput")
    whh_d = nc.dram_tensor("whh", [2, 128, 4 * H], dt.bfloat16, kind="ExternalInput")
    bias_d = nc.dram_tensor("bias", [128, 8], dt.float32, kind="ExternalInput")
    embc_d = nc.dram_tensor("embc", [2, 128, E], dt.bfloat16, kind="ExternalInput")
    iota_d = nc.dram_tensor("iota", [128, 2], dt.float32, kind="ExternalInput")
    out_d = nc.dram_tensor("out", [2, 128, C], dt.float32, kind="ExternalOutput")

    with tile.TileContext(nc) as tc:
        with (
            tc.tile_pool(name="const", bufs=1) as constp,
            tc.tile_pool(name="state", bufs=1) as statep,
            tc.tile_pool(name="chp", bufs=3) as chp,
            tc.tile_pool(name="oh", bufs=3) as ohp,
            tc.tile_pool(name="gates", bufs=2 if x_sep else 3) as gatesp,
            tc.tile_pool(name="work", bufs=2 if x_sep else 3) as workp,
            tc.tile_pool(name="psum", bufs=4, space="PSUM") as psump,
        ):
            embt_sb = constp.tile([128, V], dt.bfloat16, tag="embt")
            wih_sb = constp.tile([E, 4 * H], dt.bfloat16, tag="wih")
            whh_sb = [
                constp.tile([128, 4 * H], dt.bfloat16, tag=f"whh{p}", name=f"whh{p}")
                for p in range(2)
            ]
            bias_sb = constp.tile([128, 8], dt.float32, tag="bias")
            iota_sb = constp.tile([128, 2], dt.float32, tag="iota")
            embproj_sb = [
                constp.tile([128, 4 * H], dt.bfloat16, tag=f"ep{v}", name=f"ep{v}")
                for v in range(2)
            ]
            embc_sb = None
            xall = None
            if x_sep:
                embc_sb = [
                    constp.tile([128, E], dt.bfloat16, tag=f"ec{v}", name=f"ec{v}")
                    for v in range(2)
                ]
                for v in range(2):
                    nc.sync.dma_start(out=embc_sb[v], in_=embc_d[v])
                xoffs = np.concatenate([[0], np.cumsum(A)]).astype(int)
                xall = statep.tile([128, int(xoffs[-1]) + 128], dt.bfloat16, tag="xall")
            nc.sync.dma_start(out=embt_sb, in_=embt_d[:])
            nc.sync.dma_start(out=wih_sb, in_=wih_d[:])
            for p in range(2):
                nc.sync.dma_start(out=whh_sb[p], in_=whh_d[p])
            nc.sync.dma_start(out=bias_sb, in_=bias_d[:])
            nc.sync.dma_start(out=iota_sb, in_=iota_d[:])

            hbf = [
                statep.tile([128, C], dt.bfloat16, tag=f"h{p}", name=f"h{p}")
                for p in range(2)
            ]
            hf = [
                statep.tile([128, C], dt.float32, tag=f"hf{p}", name=f"hf{p}")
                for p in range(2)
            ]
            cst = [
                statep.tile([128, C], dt.float32, tag=f"c{p}", name=f"c{p}")
                for p in range(2)
            ]
            hdummy = None
            if nochain:
                hdummy = [
                    statep.tile([128, QW], dt.bfloat16, tag=f"hd{p}", name=f"hd{p}")
                    for p in range(2)
                ]
                for p in range(2):
                    nc.vector.memset(hdummy[p][:], 0.1)

            loop_cm = tc.For_i(0, reps, 1) if reps > 1 else contextlib.nullcontext()
            with loop_cm:
                # emb_proj[v, :] = emb[v, :] @ W_ih.T  -> 2 chunk tiles [128, 4H]
                for v in range(2) if not x_sep else []:
                    for hh in range(2):
                        pp = psump.tile([128, MM], dt.float32, tag="ps")
                        nc.tensor.matmul(
                            pp,
                            embt_sb[:, ts(v, 128)],
                            wih_sb[:, ts(hh, MM)],
                            start=True,
                            stop=True,
                        )
                        nc.scalar.copy(out=embproj_sb[v][:, ts(hh, MM)], in_=pp)

                for t in range(W):
                    At = A[t]
                    if At == 0:
                        break
                    At_next = A[t + 1] if t + 1 < W else 0
                    first = t == 0
                    ms = [0, 1, 4, 5, 6, 7] if first else list(range(8))
                    kchunks = 2 if first else 4

                    chrep = chp.tile([128, C], dt.bfloat16, tag="chrep")
                    if chrep_host:
                        nc.sync.dma_start(out=chrep[:, :At], in_=chf_d[t, :, :At])
                    else:
                        src = chf_d[t, :At]
                        nc.sync.dma_start(
                            out=chrep[:, :At],
                            in_=bass.AP(
                                tensor=src.tensor, offset=src.offset,
                                ap=[[0, 128]] + list(src.ap),
                            ),
                        )

                    nq = -(-At // QW)
                    if act_pad:
                        widths = [min(QW, At - QW * q) for q in range(nq)]
                        qlos = [QW * q for q in range(nq)]
                    else:
                        widths = []
                        rem = At
                        for q in range(nq):
                            wq = min(-(-(rem // (nq - q)) // 16) * 16, rem)
                            widths.append(wq)
                            rem -= wq
                        qlos = [int(x) for x in np.concatenate([[0], np.cumsum(widths)])[:-1]]
                    for q in range(nq):
                        qlo = int(qlos[q])
                        b = int(widths[q])
                        ohs = []
                        for v in range(2):
                            ohv = ohp.tile([128, QW], dt.bfloat16, tag=f"oh{v}", name=f"oh{v}")
                            nc.gpsimd.tensor_scalar(
                                ohv[:, :b],
                                chrep[:, qlo : qlo + b],
                                iota_sb[:, v : v + 1],
                                None,
                                op0=EQ,
                            )
                            ohs.append(ohv)
                        if x_sep:
                            xps = psump.tile([128, QW], dt.float32, tag="ps")
                            for hh in range(-(-b // mm_n)):
                                w = min(mm_n, b - mm_n * hh)
                                osl = slice(mm_n * hh, mm_n * hh + w)
                                nc.tensor.matmul(
                                    xps[:, osl], embc_sb[0], ohs[0][:, osl],
                                    start=True, stop=False,
                                )
                                nc.tensor.matmul(
                                    xps[:, osl], embc_sb[1], ohs[1][:, osl],
                                    start=False, stop=True,
                                )
                            xaw = min(QW, -(-b // 128) * 128)
                            xo = int(xoffs[t]) + qlo
                            nc.scalar.copy(
                                out=xall[:, xo : xo + xaw], in_=xps[:, :xaw]
                            )
                        gt = []
                        for m in ms:
                            ps = psump.tile([128, QW], dt.float32, tag="ps")
                            for hh in range(-(-b // mm_n)):
                                w = min(mm_n, b - mm_n * hh)
                                osl = slice(mm_n * hh, mm_n * hh + w)
                                csl = slice(qlo + mm_n * hh, qlo + mm_n * hh + w)
                                if x_sep:
                                    xo = int(xoffs[t]) + qlo
                                    nc.tensor.matmul(
                                        ps[:, osl],
                                        wih_sb[:, ts(m, 128)],
                                        xall[:, xo + mm_n * hh : xo + mm_n * hh + w],
                                        start=True,
                                        stop=first,
                                    )
                                else:
                                    nc.tensor.matmul(
                                        ps[:, osl],
                                        embproj_sb[0][:, ts(m, 128)],
                                        ohs[0][:, osl],
                                        start=True,
                                        stop=False,
                                    )
                                    nc.tensor.matmul(
                                        ps[:, osl],
                                        embproj_sb[1][:, ts(m, 128)],
                                        ohs[1][:, osl],
                                        start=False,
                                        stop=first,
                                    )
                                if not first:
                                    hs0 = hdummy[0][:, : w] if nochain else hbf[0][:, csl]
                                    hs1 = hdummy[1][:, : w] if nochain else hbf[1][:, csl]
                                    nc.tensor.matmul(
                                        ps[:, osl],
                                        whh_sb[0][:, ts(m, 128)],
                                        hs0,
                                        start=False,
                                        stop=False,
                                    )
                                    nc.tensor.matmul(
                                        ps[:, osl],
                                        whh_sb[1][:, ts(m, 128)],
                                        hs1,
                                        start=False,
                                        stop=True,
                                    )
                            gtile = gatesp.tile(
                                [128, QW], dt.bfloat16, tag=f"g{m}", name=f"g{m}"
                            )
                            if act_round:
                                aw = min(QW, -(-b // act_round) * act_round)
                            else:
                                aw = QW if act_pad else b
                            nc.scalar.activation(
                                gtile[:, :aw],
                                ps[:, :aw],
                                getattr(AF, GATE_FUNCS[m // 2]),
                                bias=bias_sb[:, m : m + 1],
                            )
                            gt.append(gtile)
                        gts = {m: g for m, g in zip(ms, gt)}
                        subw = cell_sub if cell_sub else b
                        for slo in range(0, b, subw):
                          sw = min(subw, b - slo)
                          for p in range(2):
                            csl = cst[p][:, qlo + slo : qlo + slo + sw]
                            ssl = slice(slo, slo + sw)
                            if first:
                                nc.vector.tensor_mul(
                                    csl, gts[0 + p][:, ssl], gts[4 + p][:, ssl]
                                )
                            else:
                                ig = workp.tile([128, QW], dt.float32, tag=f"ig{p}", name=f"ig{p}")
                                nc.vector.tensor_mul(
                                    ig[:, :sw], gts[0 + p][:, ssl], gts[4 + p][:, ssl]
                                )
                                fc = workp.tile([128, QW], dt.float32, tag=f"fc{p}", name=f"fc{p}")
                                nc.vector.tensor_mul(fc[:, :sw], gts[2 + p][:, ssl], csl)
                                nc.vector.tensor_add(csl, ig[:, :sw], fc[:, :sw])
                            th = workp.tile([128, QW], dt.bfloat16, tag=f"th{p}", name=f"th{p}")
                            nc.scalar.activation(th[:, :sw], csl, AF.Tanh)
                            hb = min(qlo + slo + sw, At_next) - (qlo + slo)
                            if hcopy == "gpsimd":
                                nc.vector.tensor_mul(
                                    hf[p][:, qlo + slo : qlo + slo + sw],
                                    gts[6 + p][:, ssl],
                                    th[:, :sw],
                                )
                                if hb > 0:
                                    nc.gpsimd.tensor_copy(
                                        hbf[p][:, qlo + slo : qlo + slo + hb],
                                        hf[p][:, qlo + slo : qlo + slo + hb],
                                    )
                            else:
                                if hb > 0:
                                    nc.vector.tensor_mul(
                                        hbf[p][:, qlo + slo : qlo + slo + hb],
                                        gts[6 + p][:, slo : slo + hb],
                                        th[:, :hb],
                                    )
                                nc.vector.tensor_mul(
                                    hf[p][:, qlo + slo : qlo + slo + sw],
                                    gts[6 + p][:, ssl],
                                    th[:, :sw],
                                )

                for p in range(2):
                    nc.sync.dma_start(out=out_d[p], in_=hf[p][:])

    nc.compile()
    _PROGRAM_CACHE[key] = nc
    return nc


def _prepare(char_input, embedding, W_ih, W_hh, b_ih, b_hh):
    ci = np.asarray(char_input)
    chars = ci.reshape(-1, W).astype(np.int64)
    lens = (chars != 0).sum(-1)

    colsL, C, A = _plan(lens)
    colmap, col_chars = _assign(lens, chars, colsL, C)

    embt = np.ascontiguousarray(np.asarray(embedding).T.astype(BF16))  # [128, V]
    wih_bf = np.ascontiguousarray(np.asarray(W_ih).T.astype(BF16))  # [E, 4H]
    whh_bf = np.ascontiguousarray(
        np.asarray(W_hh).T.astype(BF16).reshape(2, 128, 4 * H)
    )
    bias_h = np.ascontiguousarray(
        (np.asarray(b_ih) + np.asarray(b_hh)).astype(np.float32).reshape(8, 128).T
    )
    iota = np.ascontiguousarray(
        (np.arange(128)[:, None] + np.array([0, 128])[None, :]).astype(np.float32)
    )

    common = {
        "embt": embt,
        "wih": wih_bf,
        "whh": whh_bf,
        "bias": bias_h,
        "iota": iota,
        "embc": np.ascontiguousarray(
            np.asarray(embt).T.astype(BF16).reshape(2, 128, E)
        ),
    }
    in_maps = []
    for k in range(NCORES):
        chf = np.ascontiguousarray(col_chars[k].T.astype(BF16))  # [W, C]
        chfr = np.ascontiguousarray(np.broadcast_to(chf[:, None, :], (W, 128, chf.shape[1])))
        in_maps.append({"chf": chf, "chfr": chfr, **common})
    return colmap, in_maps, C, A


def _gather_output(results, colmap):
    out_flat = np.zeros((B * S, H), np.float32)
    for k in range(NCORES):
        o = results[k]["out"].astype(np.float32)  # [2, 128, C]
        h_core = o.reshape(H, o.shape[-1])
        mask = colmap[k] >= 0
        out_flat[colmap[k][mask]] = h_core[:, mask].T
    return out_flat.reshape(B, S, H)


def kernel(char_input, embedding, W_ih, W_hh, b_ih, b_hh):
    colmap, in_maps, C, A = _prepare(char_input, embedding, W_ih, W_hh, b_ih, b_hh)
    nc = _build_program(C, A)
    res = run_bass_kernel_spmd(nc, in_maps, core_ids=list(range(NCORES)))
    return _gather_output(res.results, colmap)


# revision 22
# speedup vs baseline: 2.6813x; 2.6813x over previous
"""Character-LSTM Trainium2 kernel (8 NeuronCores, SPMD data-parallel).

Strategy
--------
All B*S = 16384 words run one batched LSTM recurrence. Work is split across 8
cores by dealing words (sorted by descending length) round-robin so every core
sees an identical per-step active-column count A[t]; within a core, words live
in SBUF as columns of transposed state tiles [H x cols]. At step t only the
first A[t] columns are touched, so a word's last update lands exactly at its
final character and the surviving h columns are the output. Short word-length
buckets are padded with dummy columns (char 0 -> zero embedding row) so A[t]
is core-uniform and a multiple of 16.

Per step, gates are computed in transposed layout g[4H x cols] on the PE as
one accumulation over four K=128 chunks: two one-hot chunks against the
per-vocab gate table emb_proj = W_ih @ emb[v] (precomputed on device), and two
h chunks against W_hh - all bf16 with fp32 PSUM accumulation. One-hots are
built on device by GPSIMD is_equal against a per-partition iota, from a
DMA-broadcast char row. Sigmoid/tanh run on the scalar engine straight out of
1024-wide PSUM reads with the fused per-partition bias. The cell update runs
on the vector engine in fp32; h keeps an fp32 master copy (the output) and a
GPSIMD-converted bf16 copy that feeds the next step's matmuls.
"""

import sys

if "/opt/trn_rl_repo" not in sys.path:
    sys.path.insert(0, "/opt/trn_rl_repo")

import contextlib

import numpy as np
import ml_dtypes

import concourse.bass as bass
import concourse.tile as tile
from concourse import bacc, mybir
from concourse.bass import ts
from concourse.bass_utils import run_bass_kernel_spmd

BF16 = ml_dtypes.bfloat16
NCORES = 8
B, S, W, E, H, V = 64, 256, 24, 128, 256, 256
GATE_FUNCS = ["Sigmoid", "Sigmoid", "Tanh", "Sigmoid"]  # i, f, g, o per 2 chunks
QW = 1024  # PSUM tile width (2 banks); ACT reads PSUM at line rate at this width
MM = 512  # matmul moving free-dim

_PROGRAM_CACHE: dict = {}


def _plan(lens: np.ndarray):
    """Column counts per step, uniform across cores, multiples of 16."""
    wL = np.bincount(lens, minlength=W + 1)
    colsL = np.zeros(W + 1, np.int64)
    cum = 0
    for L in range(W, 0, -1):
        need = -(-int(wL[L]) // NCORES)
        newcum = -(-(cum + need) // 16) * 16
        colsL[L] = newcum - cum
        cum = newcum
    C = max(cum, 16)
    A = [int(colsL[t + 1 :].sum()) for t in range(W)]
    return colsL, C, A


def _assign(lens, chars, colsL, C):
    """Deal words into (core, column) slots, longest first."""
    order = np.argsort(-lens, kind="stable")
    wL = np.bincount(lens, minlength=W + 1)
    colmap = np.full((NCORES, C), -1, np.int64)
    col_chars = np.zeros((NCORES, C, W), np.int64)
    pos = 0
    s = 0
    for L in range(W, 0, -1):
        cnt = int(wL[L])
        if cnt:
            ids = order[pos : pos + cnt]
            pos += cnt
            k = np.arange(cnt) % NCORES
            j = s + np.arange(cnt) // NCORES
            colmap[k, j] = ids
            col_chars[k, j] = chars[ids]
        s += int(colsL[L])
    return colmap, col_chars


def _build_program(C: int, A: list[int], reps: int = 1, act_pad: bool = True, hcopy: str = 'gpsimd', cell_sub: int = 0, chrep_host: bool = False, nochain: bool = False, mm_n: int = 512, act_round: int = 0, x_sep: bool = False, oh_eng: str = 'gpsimd'):
    key = (C, tuple(A), reps, act_pad, hcopy, cell_sub, chrep_host, nochain, mm_n, act_round, x_sep, oh_eng)
    if key in _PROGRAM_CACHE:
        return _PROGRAM_CACHE[key]

    dt = mybir.dt
    AF = mybir.ActivationFunctionType
    EQ = mybir.AluOpType.is_equal
    nc = bacc.Bacc("TRN2", target_bir_lowering=False, debug=False, num_devices=NCORES)

    if chrep_host:
        chf_d = nc.dram_tensor("chfr", [W, 128, C], dt.bfloat16, kind="ExternalInput")
    else:
        chf_d = nc.dram_tensor("chf", [W, C], dt.bfloat16, kind="ExternalInput")
    embt_d = nc.dram_tensor("embt", [128, V], dt.bfloat16, kind="ExternalInput")
    wih_d = nc.dram_tensor("wih", [E, 4 * H], dt.bfloat16, kind="ExternalInput")
    whh_d = nc.dram_tensor("whh", [2, 128, 4 * H], dt.bfloat16, kind="ExternalInput")
    bias_d = nc.dram_tensor("bias", [128, 8], dt.float32, kind="ExternalInput")
    embc_d = nc.dram_tensor("embc", [2, 128, E], dt.bfloat16, kind="ExternalInput")
    iota_d = nc.dram_tensor("iota", [128, 2], dt.float32, kind="ExternalInput")
    out_d = nc.dram_tensor("out", [2, 128, C], dt.float32, kind="ExternalOutput")

    with tile.TileContext(nc) as tc:
        with (
            tc.tile_pool(name="const", bufs=1) as constp,
            tc.tile_pool(name="state", bufs=1) as statep,
            tc.tile_pool(name="chp", bufs=3) as chp,
            tc.tile_pool(name="oh", bufs=3) as ohp,
            tc.tile_pool(name="gates", bufs=2 if x_sep else 3) as gatesp,
            tc.tile_pool(name="work", bufs=2 if x_sep else 3) as workp,
            tc.tile_pool(name="psum", bufs=4, space="PSUM") as psump,
        ):
            embt_sb = constp.tile([128, V], dt.bfloat16, tag="embt")
            wih_sb = constp.tile([E, 4 * H], dt.bfloat16, tag="wih")
            whh_sb = [
                constp.tile([128, 4 * H], dt.bfloat16, tag=f"whh{p}", name=f"whh{p}")
                for p in range(2)
            ]
            bias_sb = constp.tile([128, 8], dt.float32, tag="bias")
            iota_sb = constp.tile([128, 2], dt.float32, tag="iota")
            embproj_sb = [
                constp.tile([128, 4 * H], dt.bfloat16, tag=f"ep{v}", name=f"ep{v}")
                for v in range(2)
            ]
            embc_sb = None
            xall = None
            if x_sep:
                embc_sb = [
                    constp.tile([128, E], dt.bfloat16, tag=f"ec{v}", name=f"ec{v}")
                    for v in range(2)
                ]
                for v in range(2):
                    nc.sync.dma_start(out=embc_sb[v], in_=embc_d[v])
                xoffs = np.concatenate([[0], np.cumsum(A)]).astype(int)
                xall = statep.tile([128, int(xoffs[-1]) + 128], dt.bfloat16, tag="xall")
            nc.sync.dma_start(out=embt_sb, in_=embt_d[:])
            nc.sync.dma_start(out=wih_sb, in_=wih_d[:])
            for p in range(2):
                nc.sync.dma_start(out=whh_sb[p], in_=whh_d[p])
            nc.sync.dma_start(out=bias_sb, in_=bias_d[:])
            nc.sync.dma_start(out=iota_sb, in_=iota_d[:])

            hbf = [
                statep.tile([128, C], dt.bfloat16, tag=f"h{p}", name=f"h{p}")
                for p in range(2)
            ]
            hf = [
                statep.tile([128, C], dt.float32, tag=f"hf{p}", name=f"hf{p}")
                for p in range(2)
            ]
            cst = [
                statep.tile([128, C], dt.float32, tag=f"c{p}", name=f"c{p}")
                for p in range(2)
            ]
            hdummy = None
            if nochain:
                hdummy = [
                    statep.tile([128, QW], dt.bfloat16, tag=f"hd{p}", name=f"hd{p}")
                    for p in range(2)
                ]
                for p in range(2):
                    nc.vector.memset(hdummy[p][:], 0.1)

            loop_cm = tc.For_i(0, reps, 1) if reps > 1 else contextlib.nullcontext()
            with loop_cm:
                # emb_proj[v, :] = emb[v, :] @ W_ih.T  -> 2 chunk tiles [128, 4H]
                for v in range(2) if not x_sep else []:
                    for hh in range(2):
                        pp = psump.tile([128, MM], dt.float32, tag="ps")
                        nc.tensor.matmul(
                            pp,
                            embt_sb[:, ts(v, 128)],
                            wih_sb[:, ts(hh, MM)],
                            start=True,
                            stop=True,
                        )
                        nc.scalar.copy(out=embproj_sb[v][:, ts(hh, MM)], in_=pp)

                for t in range(W):
                    At = A[t]
                    if At == 0:
                        break
                    At_next = A[t + 1] if t + 1 < W else 0
                    first = t == 0
                    ms = [0, 1, 4, 5, 6, 7] if first else list(range(8))
                    kchunks = 2 if first else 4

                    chrep = chp.tile([128, C], dt.bfloat16, tag="chrep")
                    if chrep_host:
                        nc.sync.dma_start(out=chrep[:, :At], in_=chf_d[t, :, :At])
                    else:
                        src = chf_d[t, :At]
                        nc.sync.dma_start(
                            out=chrep[:, :At],
                            in_=bass.AP(
                                tensor=src.tensor, offset=src.offset,
                                ap=[[0, 128]] + list(src.ap),
                            ),
                        )

                    nq = -(-At // QW)
                    if act_pad:
                        widths = [min(QW, At - QW * q) for q in range(nq)]
                        qlos = [QW * q for q in range(nq)]
                    else:
                        widths = []
                        rem = At
                        for q in range(nq):
                            wq = min(-(-(rem // (nq - q)) // 16) * 16, rem)
                            widths.append(wq)
                            rem -= wq
                        qlos = [int(x) for x in np.concatenate([[0], np.cumsum(widths)])[:-1]]
                    for q in range(nq):
                        qlo = int(qlos[q])
                        b = int(widths[q])
                        ohs = []
                        for v in range(2):
                            ohv = ohp.tile([128, QW], dt.bfloat16, tag=f"oh{v}", name=f"oh{v}")
                            oh_e = nc.gpsimd if oh_eng == "gpsimd" else nc.vector
                            oh_e.tensor_scalar(
                                ohv[:, :b],
                                chrep[:, qlo : qlo + b],
                                iota_sb[:, v : v + 1],
                                None,
                                op0=EQ,
                            )
                            ohs.append(ohv)
                        if x_sep:
                            xps = psump.tile([128, QW], dt.float32, tag="ps")
                            for hh in range(-(-b // mm_n)):
                                w = min(mm_n, b - mm_n * hh)
                                osl = slice(mm_n * hh, mm_n * hh + w)
                                nc.tensor.matmul(
                                    xps[:, osl], embc_sb[0], ohs[0][:, osl],
                                    start=True, stop=False,
                                )
                                nc.tensor.matmul(
                                    xps[:, osl], embc_sb[1], ohs[1][:, osl],
                                    start=False, stop=True,
                                )
                            xaw = min(QW, -(-b // 128) * 128)
                            xo = int(xoffs[t]) + qlo
                            nc.scalar.copy(
                                out=xall[:, xo : xo + xaw], in_=xps[:, :xaw]
                            )
                        gt = []
                        for m in ms:
                            ps = psump.tile([128, QW], dt.float32, tag="ps")
                            for hh in range(-(-b // mm_n)):
                                w = min(mm_n, b - mm_n * hh)
                                osl = slice(mm_n * hh, mm_n * hh + w)
                                csl = slice(qlo + mm_n * hh, qlo + mm_n * hh + w)
                                if x_sep:
                                    xo = int(xoffs[t]) + qlo
                                    nc.tensor.matmul(
                                        ps[:, osl],
                                        wih_sb[:, ts(m, 128)],
                                        xall[:, xo + mm_n * hh : xo + mm_n * hh + w],
                                        start=True,
                                        stop=first,
                                    )
                                else:
                                    nc.tensor.matmul(
                                        ps[:, osl],
                                        embproj_sb[0][:, ts(m, 128)],
                                        ohs[0][:, osl],
                                        start=True,
                                        stop=False,
                                    )
                                    nc.tensor.matmul(
                                        ps[:, osl],
                                        embproj_sb[1][:, ts(m, 128)],
                                        ohs[1][:, osl],
                                        start=False,
                                        stop=first,
                                    )
                                if not first:
                                    hs0 = hdummy[0][:, : w] if nochain else hbf[0][:, csl]
                                    hs1 = hdummy[1][:, : w] if nochain else hbf[1][:, csl]
                                    nc.tensor.matmul(
                                        ps[:, osl],
                                        whh_sb[0][:, ts(m, 128)],
                                        hs0,
                                        start=False,
                                        stop=False,
                                    )
                                    nc.tensor.matmul(
                                        ps[:, osl],
                                        whh_sb[1][:, ts(m, 128)],
                                        hs1,
                                        start=False,
                                        stop=True,
                                    )
                            gtile = gatesp.tile(
                                [128, QW], dt.bfloat16, tag=f"g{m}", name=f"g{m}"
                            )
                            if act_round:
                                aw = min(QW, -(-b // act_round) * act_round)
                            else:
                                aw = QW if act_pad else b
                            nc.scalar.activation(
                                gtile[:, :aw],
                                ps[:, :aw],
                                getattr(AF, GATE_FUNCS[m // 2]),
                                bias=bias_sb[:, m : m + 1],
                            )
                            gt.append(gtile)
                        gts = {m: g for m, g in zip(ms, gt)}
                        subw = cell_sub if cell_sub else b
                        for slo in range(0, b, subw):
                          sw = min(subw, b - slo)
                          for p in range(2):
                            csl = cst[p][:, qlo + slo : qlo + slo + sw]
                            ssl = slice(slo, slo + sw)
                            if first:
                                nc.vector.tensor_mul(
                                    csl, gts[0 + p][:, ssl], gts[4 + p][:, ssl]
                                )
                            else:
                                ig = workp.tile([128, QW], dt.float32, tag=f"ig{p}", name=f"ig{p}")
                                nc.vector.tensor_mul(
                                    ig[:, :sw], gts[0 + p][:, ssl], gts[4 + p][:, ssl]
                                )
                                fc = workp.tile([128, QW], dt.float32, tag=f"fc{p}", name=f"fc{p}")
                                nc.vector.tensor_mul(fc[:, :sw], gts[2 + p][:, ssl], csl)
                                nc.vector.tensor_add(csl, ig[:, :sw], fc[:, :sw])
                            th = workp.tile([128, QW], dt.bfloat16, tag=f"th{p}", name=f"th{p}")
                            nc.scalar.activation(th[:, :sw], csl, AF.Tanh)
                            hb = min(qlo + slo + sw, At_next) - (qlo + slo)
                            if hcopy == "gpsimd":
                                nc.vector.tensor_mul(
                                    hf[p][:, qlo + slo : qlo + slo + sw],
                                    gts[6 + p][:, ssl],
                                    th[:, :sw],
                                )
                                if hb > 0:
                                    nc.gpsimd.tensor_copy(
                                        hbf[p][:, qlo + slo : qlo + slo + hb],
                                        hf[p][:, qlo + slo : qlo + slo + hb],
                                    )
                            else:
                                if hb > 0:
                                    nc.vector.tensor_mul(
                                        hbf[p][:, qlo + slo : qlo + slo + hb],
                                        gts[6 + p][:, slo : slo + hb],
                                        th[:, :hb],
                                    )
                                nc.vector.tensor_mul(
                                    hf[p][:, qlo + slo : qlo + slo + sw],
                                    gts[6 + p][:, ssl],
                                    th[:, :sw],
                                )

                for p in range(2):
                    nc.sync.dma_start(out=out_d[p], in_=hf[p][:])

    nc.compile()
    _PROGRAM_CACHE[key] = nc
    return nc


def _prepare(char_input, embedding, W_ih, W_hh, b_ih, b_hh):
    ci = np.asarray(char_input)
    chars = ci.reshape(-1, W).astype(np.int64)
    lens = (chars != 0).sum(-1)

    colsL, C, A = _plan(lens)
    colmap, col_chars = _assign(lens, chars, colsL, C)

    embt = np.ascontiguousarray(np.asarray(embedding).T.astype(BF16))  # [128, V]
    wih_bf = np.ascontiguousarray(np.asarray(W_ih).T.astype(BF16))  # [E, 4H]
    whh_bf = np.ascontiguousarray(
        np.asarray(W_hh).T.astype(BF16).reshape(2, 128, 4 * H)
    )
    bias_h = np.ascontiguousarray(
        (np.asarray(b_ih) + np.asarray(b_hh)).astype(np.float32).reshape(8, 128).T
    )
    iota = np.ascontiguousarray(
        (np.arange(128)[:, None] + np.array([0, 128])[None, :]).astype(np.float32)
    )

    common = {
        "embt": embt,
        "wih": wih_bf,
        "whh": whh_bf,
        "bias": bias_h,
        "iota": iota,
        "embc": np.ascontiguousarray(
            np.asarray(embt).T.astype(BF16).reshape(2, 128, E)
        ),
    }
    in_maps = []
    for k in range(NCORES):
        chf = np.ascontiguousarray(col_chars[k].T.astype(BF16))  # [W, C]
        chfr = np.ascontiguousarray(np.broadcast_to(chf[:, None, :], (W, 128, chf.shape[1])))
        in_maps.append({"chf": chf, "chfr": chfr, **common})
    return colmap, in_maps, C, A


def _gather_output(results, colmap):
    out_flat = np.zeros((B * S, H), np.float32)
    for k in range(NCORES):
        o = results[k]["out"].astype(np.float32)  # [2, 128, C]
        h_core = o.reshape(H, o.shape[-1])
        mask = colmap[k] >= 0
        out_flat[colmap[k][mask]] = h_core[:, mask].T
    return out_flat.reshape(B, S, H)


def kernel(char_input, embedding, W_ih, W_hh, b_ih, b_hh):
    colmap, in_maps, C, A = _prepare(char_input, embedding, W_ih, W_hh, b_ih, b_hh)
    nc = _build_program(C, A)
    res = run_bass_kernel_spmd(nc, in_maps, core_ids=list(range(NCORES)))
    return _gather_output(res.results, colmap)
